# revision 17
# baseline (speedup 1.0000x reference)
"""Trainium2 Bass kernel for CrossModalFusion (B=4, C=64, H=W=64, N=4096).

Reference computation (per sample b, with x reshaped to [C, N]):
    q = wq @ xo + bq          [8, N]
    k = wk @ xs + bk          [8, N]
    v = wv @ xs + bv          [64, N]
    S[n, m]  = q[:, n] . k[:, m]
    attn     = softmax_m(S)
    out      = gamma * (v @ attn^T) + x_opt

Sharding: 8 cores = 4 batch samples x 2 halves of the query (n) axis.
Each core computes output rows [64, 2048] for its (sample, n-half); no
cross-core communication is needed.

Per-core dataflow (bf16 fast path):
  - biases are folded into augmented weights on the host (ones-row trick);
    gamma is folded into the v columns of wv (the denominator column stays
    unscaled, so gamma cancels out of the normalization exactly when the
    kernel divides by the accumulated denominator).
  - all PE-heavy matmuls run in bf16 (observed ~3x faster per 512-col
    matmul than fp32r on TRN2): q/k/v projections, S^T score matmuls and
    the attention*V accumulation.  Scores accumulate in fp32 PSUM; exp
    runs on the scalar (ACT) engine reading PSUM and writing bf16 SBUF.
  - scores are computed TRANSPOSED (S^T[m, n]) so the exp'd scores feed
    the attention*V matmul directly as the moving operand.  v^T carries an
    extra ones column, so the AV matmul's row 64 accumulates
    sum_m exp(S[n, m]) -- the softmax denominator comes out of the same
    accumulation for free.  No max-subtraction is needed: scores are O(3).
  - q/k are replicated at partition offsets 0 and 64 so the rank-8 S^T
    matmuls alternate PE row groups (stationary double-buffering).
  - per n-tile of 512: accumulate all 32 m-blocks, then normalize via
    reciprocal_approx_fast (custom DVE op, ~5x faster than the iterative
    divide), a PE ones-broadcast matmul, and two DVE element-wise ops.
    The normalize for tile t is emitted in the middle of tile t+1's wave
    loop so it never stalls the PE at tile boundaries.
  - residual x_opt is DMA'd separately in fp32 (off the critical path) so
    the gamma=0 output is bit-accurate to x_opt up to the fp32 add.
"""

import os
import sys

import numpy as np

for _p in ("/opt/trn_rl_repo", "/root/.axon_site/_ro/trn_rl_repo"):
    if os.path.isdir(_p) and _p not in sys.path:
        sys.path.insert(0, _p)

import ml_dtypes

import concourse.bass as bass
import concourse.mybir as mybir
import concourse.tile as tile
from concourse import bacc
from concourse.bass_utils import run_bass_kernel_spmd

F32 = mybir.dt.float32
F32R = mybir.dt.float32r
BF16 = mybir.dt.bfloat16
AF = mybir.ActivationFunctionType
NP_BF16 = np.dtype(ml_dtypes.bfloat16)

B, C, HH, WW = 4, 64, 64, 64
N = HH * WW            # 4096 key/query positions
D = 8                  # q/k channel count
CA = C + 1             # augmented channel dim (ones row / denominator row)
VW = CA + 1            # padded v^T block width (66: even col count for bf16)
WCOLS = D + D + VW     # packed weight buffer width (wq | wk | wv')
NCORES = 8
NL = N // 2            # query rows per core
NT = 512               # n-tile (PSUM bank width in fp32)
MB = 128               # m-block (PE partition width)
N_NT = NL // NT        # 4 n-tiles per core
N_MB = N // MB         # 32 m-blocks
WAVE = 2               # m-blocks exp'd per ACT instruction


def build_program(repeat: int = 1) -> bass.Bass:
    # Bacc (not raw Bass): its compile() pass splits multi-semaphore waits
    # and moves matmul waits onto LDWEIGHTS, which this walrus build requires.
    nc = bacc.Bacc("TRN2", target_bir_lowering=False, num_devices=NCORES)
    xo_d = nc.declare_dram_parameter("xo_bf", [CA, NL], BF16, isOutput=False)
    xs_d = nc.declare_dram_parameter("xs_bf", [CA, N], BF16, isOutput=False)
    xof_d = nc.declare_dram_parameter("xof", [C, NL], F32, isOutput=False)
    w_d = nc.declare_dram_parameter("wpack", [CA, WCOLS], BF16, isOutput=False)
    out_d = nc.declare_dram_parameter("out", [C, NL], F32, isOutput=True)

    with tile.TileContext(nc) as tc:
      for _rep in range(repeat):
        with tc.tile_pool(name="const", bufs=1) as cp:
            # --- input DMAs, spread across per-engine DGE queues ---
            w_sb = cp.tile([CA, WCOLS], BF16)
            nc.sync.dma_start(w_sb[:], w_d[:])
            xo_bf = cp.tile([CA, NL], BF16)
            for j in range(2):
                nc.sync.dma_start(
                    xo_bf[:, j * 1024 : (j + 1) * 1024],
                    xo_d[:, j * 1024 : (j + 1) * 1024],
                )
            xs_bf = cp.tile([CA, N], BF16)
            for j in range(4):
                nc.gpsimd.dma_start(
                    xs_bf[:, j * 1024 : (j + 1) * 1024],
                    xs_d[:, j * 1024 : (j + 1) * 1024],
                )
            xof_sb = cp.tile([C, NL], F32)
            wq_sb = w_sb[:, 0:D]
            wk_sb = w_sb[:, D : 2 * D]
            wv_sb = w_sb[:, 2 * D : 2 * D + VW]

            # q/k at partition offsets 0 and 64 (PE row groups for the
            # alternating rank-8 S^T matmuls); v^T with denominator column.
            q_rep = cp.tile([64 + D, NL], BF16)
            k_rep = cp.tile([64 + D, N], BF16)
            vT = cp.tile([MB, N_MB * VW], BF16)

            with tc.tile_pool(name="pre_ps", bufs=3, space="PSUM") as pp:
                def emit_q(j):
                    qp = pp.tile([D, NT], F32, tag="qk_ps")
                    nc.tensor.matmul(
                        qp[:], wq_sb, xo_bf[:, j * NT : (j + 1) * NT],
                        start=True, stop=True,
                    )
                    sl = q_rep[0:D, j * NT : (j + 1) * NT]
                    # q casts on ACT (idle in preamble), k casts on DVE
                    nc.scalar.copy(sl, qp[:])
                    # partition-offset replica: engines are lane-aligned, so
                    # this must be a DMA (gpsimd DGE queue)
                    nc.gpsimd.dma_start(q_rep[64 : 64 + D, j * NT : (j + 1) * NT], sl)

                def emit_k(j):
                    kp = pp.tile([D, NT], F32, tag="qk_ps")
                    nc.tensor.matmul(
                        kp[:], wk_sb, xs_bf[:, j * NT : (j + 1) * NT],
                        start=True, stop=True,
                    )
                    sl = k_rep[0:D, j * NT : (j + 1) * NT]
                    nc.vector.tensor_copy(sl, kp[:])
                    nc.gpsimd.dma_start(k_rep[64 : 64 + D, j * NT : (j + 1) * NT], sl)

                def emit_v(g):
                    # 4 m-blocks per PSUM tile, one cast for all 4
                    vp = pp.tile([MB, 4 * VW], F32, tag="vp_ps")
                    for t in range(4):
                        mb = 4 * g + t
                        nc.tensor.matmul(
                            vp[:, t * VW : (t + 1) * VW],
                            xs_bf[:, mb * MB : (mb + 1) * MB],
                            wv_sb,
                            start=True, stop=True,
                        )
                    nc.vector.tensor_copy(
                        vT[:, g * 4 * VW : (g + 1) * 4 * VW], vp[:]
                    )

                # ordered by first use in the wave loop
                emit_q(0); emit_q(1); emit_k(0); emit_k(1)
                emit_v(0); emit_v(1)
                emit_q(2); emit_q(3); emit_k(2); emit_k(3)
                emit_v(2); emit_v(3)
                emit_k(4); emit_k(5); emit_v(4); emit_v(5)
                emit_k(6); emit_k(7); emit_v(6); emit_v(7)

            # residual; needed late (first normalize), queued after replicas
            for j in range(2):
                nc.gpsimd.dma_start(
                    xof_sb[:, j * 1024 : (j + 1) * 1024],
                    xof_d[:, j * 1024 : (j + 1) * 1024],
                )

            with (
                tc.tile_pool(name="st_ps", bufs=3, space="PSUM") as st_pool,
                tc.tile_pool(name="av_ps", bufs=2, space="PSUM") as av_pool,
                tc.tile_pool(name="e_sb", bufs=6) as e_pool,
                tc.tile_pool(name="o_sb", bufs=2) as o_pool,
                tc.tile_pool(name="sm_sb", bufs=2) as sm_pool,
            ):
                def norm_recip(av, last):
                    # softmax denominator -> reciprocal (row 64 of av).
                    # Last tile is on the drain critical path: exp(-ln(d)) on
                    # the (by then idle) ACT engine is ~3x faster than the
                    # iterative DVE divide; Ln and Exp share one table set.
                    r = sm_pool.tile([1, NT], F32, tag="r")
                    if last:
                        lt = sm_pool.tile([1, NT], F32, tag="lt")
                        nc.scalar.activation(lt[:], av[C:CA, :], AF.Ln)
                        nc.scalar.activation(r[:], lt[:], AF.Exp, scale=-1.0)
                    else:
                        nc.vector.reciprocal(r[:], av[C:CA, :])
                    return r

                def norm_apply(av, r, nt):
                    n0, n1 = nt * NT, (nt + 1) * NT
                    # broadcast 1/denom across the 64 channel partitions via
                    # a stride-0 DMA (no PE/DVE time)
                    bcs = o_pool.tile([C, NT], F32, tag="bcs")
                    nc.gpsimd.partition_broadcast(bcs[:], r[:])
                    om = o_pool.tile([C, NT], F32, tag="om")
                    nc.vector.tensor_mul(om[:], av[0:C, :], bcs[:])
                    o = o_pool.tile([C, NT], F32, tag="o")
                    nc.gpsimd.tensor_add(o[:], om[:], xof_sb[:, n0:n1])
                    nc.sync.dma_start(out_d[:, n0:n1], o[:])

                prev = None  # (av, r, nt) awaiting apply
                for nt in range(N_NT):
                    n0, n1 = nt * NT, (nt + 1) * NT
                    av = av_pool.tile([CA, NT], F32)

                    def emit_av(e_t, w, av=av):
                        for j in range(WAVE):
                            mb = WAVE * w + j
                            nc.tensor.matmul(
                                av[:],
                                vT[:, mb * VW : mb * VW + CA],
                                e_t[:, j * NT : (j + 1) * NT],
                                start=(mb == 0),
                                stop=(mb == N_MB - 1),
                            )

                    # S^T matmuls + exp, with the AV accumulation lagging two
                    # waves so the PE never stalls waiting on the current exp.
                    pend = []
                    for w in range(N_MB // WAVE):
                        st = st_pool.tile([MB, WAVE * NT], F32)
                        for j in range(WAVE):
                            mb = WAVE * w + j
                            rg = 64 * j
                            nc.tensor.matmul(
                                st[:, j * NT : (j + 1) * NT],
                                k_rep[rg : rg + D, mb * MB : (mb + 1) * MB],
                                q_rep[rg : rg + D, n0:n1],
                                start=True,
                                stop=True,
                            )
                        e_t = e_pool.tile([MB, WAVE * NT], BF16)
                        nc.scalar.activation(e_t[:], st[:], AF.Exp)
                        pend.append((e_t, w))
                        if len(pend) > 2:
                            emit_av(*pend.pop(0))
                        # normalize of the PREVIOUS tile, mid-stream so the
                        # PE/DVE chain never gates a tile boundary
                        if w == 2 and prev is not None:
                            norm_apply(*prev)
                            prev = None
                    for p in pend:
                        emit_av(*p)
                    prev = (av, norm_recip(av, nt == N_NT - 1), nt)

                norm_apply(*prev)
    nc.compile()
    return nc


_NC = None


def _get_nc() -> bass.Bass:
    global _NC
    if _NC is None:
        _NC = build_program()
    return _NC


def make_in_maps(x_opt, x_sar, wq, bq, wk, bk, wv, bv, gamma):
    f = np.float32
    x_opt = np.asarray(x_opt, f).reshape(B, C, N)
    x_sar = np.asarray(x_sar, f).reshape(B, C, N)
    g = float(np.asarray(gamma, f).reshape(-1)[0])

    wq_aug = np.concatenate([np.asarray(wq, f).T, np.asarray(bq, f)[None, :]], 0)
    wk_aug = np.concatenate([np.asarray(wk, f).T, np.asarray(bk, f)[None, :]], 0)
    wv_aug = np.zeros((CA, VW), f)
    wv_aug[:C, :C] = np.asarray(wv, f).T * g
    wv_aug[C, :C] = np.asarray(bv, f) * g
    wv_aug[C, C] = 1.0  # denominator column (gamma cancels in the divide)
    wpack = np.ascontiguousarray(
        np.concatenate([wq_aug, wk_aug, wv_aug], axis=1).astype(NP_BF16)
    )

    ones_n = np.ones((1, N), f)
    maps = []
    for core in range(NCORES):
        b, h = divmod(core, 2)
        xo = x_opt[b, :, h * NL : (h + 1) * NL]
        xo_bf = np.ascontiguousarray(
            np.concatenate([xo, ones_n[:, :NL]], axis=0).astype(NP_BF16)
        )
        xs_bf = np.ascontiguousarray(
            np.concatenate([x_sar[b], ones_n], axis=0).astype(NP_BF16)
        )
        maps.append(
            {
                "xo_bf": xo_bf,
                "xs_bf": xs_bf,
                "xof": np.ascontiguousarray(xo),
                "wpack": wpack,
            }
        )
    return maps


def assemble_out(results) -> np.ndarray:
    out = np.empty((B, C, N), np.float32)
    for core in range(NCORES):
        b, h = divmod(core, 2)
        out[b, :, h * NL : (h + 1) * NL] = results[core]["out"]
    return out.reshape(B, C, HH, WW)


def kernel(**inputs) -> np.ndarray:
    nc = _get_nc()
    maps = make_in_maps(**inputs)
    res = run_bass_kernel_spmd(nc, maps, list(range(NCORES)))
    return assemble_out(res.results)


# revision 22
# speedup vs baseline: 1.0415x; 1.0415x over previous
"""Trainium2 Bass kernel for CrossModalFusion (B=4, C=64, H=W=64, N=4096).

Reference computation (per sample b, with x reshaped to [C, N]):
    q = wq @ xo + bq          [8, N]
    k = wk @ xs + bk          [8, N]
    v = wv @ xs + bv          [64, N]
    S[n, m]  = q[:, n] . k[:, m]
    attn     = softmax_m(S)
    out      = gamma * (v @ attn^T) + x_opt

Sharding: 8 cores = 4 batch samples x 2 halves of the query (n) axis.
Each core computes output rows [64, 2048] for its (sample, n-half); no
cross-core communication is needed.

Per-core dataflow (bf16 fast path):
  - biases are folded into augmented weights on the host (ones-row trick);
    gamma is folded into the v columns of wv (the denominator column stays
    unscaled, so gamma cancels out of the normalization exactly when the
    kernel divides by the accumulated denominator).
  - all PE-heavy matmuls run in bf16 (observed ~3x faster per 512-col
    matmul than fp32r on TRN2): q/k/v projections, S^T score matmuls and
    the attention*V accumulation.  Scores accumulate in fp32 PSUM; exp
    runs on the scalar (ACT) engine reading PSUM and writing bf16 SBUF.
  - scores are computed TRANSPOSED (S^T[m, n]) so the exp'd scores feed
    the attention*V matmul directly as the moving operand.  v^T carries an
    extra ones column, so the AV matmul's row 64 accumulates
    sum_m exp(S[n, m]) -- the softmax denominator comes out of the same
    accumulation for free.  No max-subtraction is needed: scores are O(3).
  - q/k are replicated at partition offsets 0 and 64 so the rank-8 S^T
    matmuls alternate PE row groups (stationary double-buffering).
  - per n-tile of 512: accumulate all 32 m-blocks, then normalize via
    reciprocal_approx_fast (custom DVE op, ~5x faster than the iterative
    divide), a PE ones-broadcast matmul, and two DVE element-wise ops.
    The normalize for tile t is emitted in the middle of tile t+1's wave
    loop so it never stalls the PE at tile boundaries.
  - residual x_opt is DMA'd separately in fp32 (off the critical path) so
    the gamma=0 output is bit-accurate to x_opt up to the fp32 add.
"""

import os
import sys

import numpy as np

for _p in ("/opt/trn_rl_repo", "/root/.axon_site/_ro/trn_rl_repo"):
    if os.path.isdir(_p) and _p not in sys.path:
        sys.path.insert(0, _p)

import ml_dtypes

import concourse.bass as bass
import concourse.mybir as mybir
import concourse.tile as tile
from concourse import bacc
from concourse.bass_utils import run_bass_kernel_spmd

F32 = mybir.dt.float32
F32R = mybir.dt.float32r
BF16 = mybir.dt.bfloat16
AF = mybir.ActivationFunctionType
NP_BF16 = np.dtype(ml_dtypes.bfloat16)

B, C, HH, WW = 4, 64, 64, 64
N = HH * WW            # 4096 key/query positions
D = 8                  # q/k channel count
CA = C + 1             # augmented channel dim (ones row / denominator row)
VW = CA + 1            # padded v^T block width (66: even col count for bf16)
WCOLS = D + D + VW     # packed weight buffer width (wq | wk | wv')
NCORES = 8
NL = N // 2            # query rows per core
NT = 512               # n-tile (PSUM bank width in fp32)
MB = 128               # m-block (PE partition width)
N_NT = NL // NT        # 4 n-tiles per core
N_MB = N // MB         # 32 m-blocks
WAVE = 2               # m-blocks exp'd per ACT instruction


def build_program(repeat: int = 1) -> bass.Bass:
    # Bacc (not raw Bass): its compile() pass splits multi-semaphore waits
    # and moves matmul waits onto LDWEIGHTS, which this walrus build requires.
    nc = bacc.Bacc("TRN2", target_bir_lowering=False, num_devices=NCORES)
    xo_d = nc.declare_dram_parameter("xo_bf", [CA, NL], BF16, isOutput=False)
    xs_d = nc.declare_dram_parameter("xs_bf", [CA, N], BF16, isOutput=False)
    xof_d = nc.declare_dram_parameter("xof", [C, NL], F32, isOutput=False)
    w_d = nc.declare_dram_parameter("wpack", [CA, WCOLS], BF16, isOutput=False)
    out_d = nc.declare_dram_parameter("out", [C, NL], F32, isOutput=True)

    with tile.TileContext(nc) as tc:
      for _rep in range(repeat):
        with tc.tile_pool(name="const", bufs=1) as cp:
            # --- input DMAs, spread across per-engine DGE queues ---
            w_sb = cp.tile([CA, WCOLS], BF16)
            nc.sync.dma_start(w_sb[:], w_d[:])
            xo_bf = cp.tile([CA, NL], BF16)
            for j in range(2):
                nc.sync.dma_start(
                    xo_bf[:, j * 1024 : (j + 1) * 1024],
                    xo_d[:, j * 1024 : (j + 1) * 1024],
                )
            xs_bf = cp.tile([CA, N], BF16)
            for j in range(4):
                nc.gpsimd.dma_start(
                    xs_bf[:, j * 1024 : (j + 1) * 1024],
                    xs_d[:, j * 1024 : (j + 1) * 1024],
                )
            xof_sb = cp.tile([C, NL], F32)
            ones_sb = cp.tile([1, C], BF16)
            nc.vector.memset(ones_sb[:], 1.0)
            wq_sb = w_sb[:, 0:D]
            wk_sb = w_sb[:, D : 2 * D]
            wv_sb = w_sb[:, 2 * D : 2 * D + VW]

            # q/k at partition offsets 0 and 64 (PE row groups for the
            # alternating rank-8 S^T matmuls); v^T with denominator column.
            q_rep = cp.tile([64 + D, NL], BF16)
            k_rep = cp.tile([64 + D, N], BF16)
            vT = cp.tile([MB, N_MB * VW], BF16)

            with tc.tile_pool(name="pre_ps", bufs=3, space="PSUM") as pp:
                def emit_q(j):
                    qp = pp.tile([D, NT], F32, tag="qk_ps")
                    nc.tensor.matmul(
                        qp[:], wq_sb, xo_bf[:, j * NT : (j + 1) * NT],
                        start=True, stop=True,
                    )
                    sl = q_rep[0:D, j * NT : (j + 1) * NT]
                    # q casts on ACT (idle in preamble), k casts on DVE
                    nc.scalar.copy(sl, qp[:])
                    # partition-offset replica: engines are lane-aligned, so
                    # this must be a DMA (gpsimd DGE queue)
                    nc.gpsimd.dma_start(q_rep[64 : 64 + D, j * NT : (j + 1) * NT], sl)

                def emit_k(j):
                    kp = pp.tile([D, NT], F32, tag="qk_ps")
                    nc.tensor.matmul(
                        kp[:], wk_sb, xs_bf[:, j * NT : (j + 1) * NT],
                        start=True, stop=True,
                    )
                    sl = k_rep[0:D, j * NT : (j + 1) * NT]
                    nc.vector.tensor_copy(sl, kp[:])
                    nc.gpsimd.dma_start(k_rep[64 : 64 + D, j * NT : (j + 1) * NT], sl)

                def emit_v(g):
                    # 4 m-blocks per PSUM tile, one cast for all 4
                    vp = pp.tile([MB, 4 * VW], F32, tag="vp_ps")
                    for t in range(4):
                        mb = 4 * g + t
                        nc.tensor.matmul(
                            vp[:, t * VW : (t + 1) * VW],
                            xs_bf[:, mb * MB : (mb + 1) * MB],
                            wv_sb,
                            start=True, stop=True,
                        )
                    nc.vector.tensor_copy(
                        vT[:, g * 4 * VW : (g + 1) * 4 * VW], vp[:]
                    )

                # ordered by first use in the wave loop
                emit_q(0); emit_q(1); emit_k(0); emit_k(1)
                emit_v(0); emit_v(1)
                emit_q(2); emit_q(3); emit_k(2); emit_k(3)
                emit_v(2); emit_v(3)
                emit_k(4); emit_k(5); emit_v(4); emit_v(5)
                emit_k(6); emit_k(7); emit_v(6); emit_v(7)

            # residual; needed late (first normalize), queued after replicas
            for j in range(2):
                nc.gpsimd.dma_start(
                    xof_sb[:, j * 1024 : (j + 1) * 1024],
                    xof_d[:, j * 1024 : (j + 1) * 1024],
                )

            with (
                tc.tile_pool(name="st_ps", bufs=2, space="PSUM") as st_pool,
                tc.tile_pool(name="av_ps", bufs=2, space="PSUM") as av_pool,
                tc.tile_pool(name="bc_ps", bufs=1, space="PSUM") as bc_pool,
                tc.tile_pool(name="e_sb", bufs=6) as e_pool,
                tc.tile_pool(name="o_sb", bufs=2) as o_pool,
                tc.tile_pool(name="sm_sb", bufs=2) as sm_pool,
            ):
                def norm_recip(av):
                    # softmax denominator -> reciprocal (row 64 of av)
                    r = sm_pool.tile([1, NT], F32, tag="r")
                    nc.vector.reciprocal(r[:], av[C:CA, :])
                    # bf16 copy feeds the broadcast matmul (bf16 is ~2x
                    # faster than the fp32 LOW/HIGH pair on the drain path)
                    rb = sm_pool.tile([1, NT], BF16, tag="rb")
                    nc.vector.tensor_copy(rb[:], r[:])
                    return rb

                def norm_apply(av, rb, nt):
                    n0, n1 = nt * NT, (nt + 1) * NT
                    # broadcast 1/denom across the 64 channel partitions via
                    # a rank-1 ones matmul
                    bc = bc_pool.tile([C, NT], F32)
                    nc.tensor.matmul(bc[:], ones_sb[:], rb[:], start=True, stop=True)
                    bcs = o_pool.tile([C, NT], F32, tag="bcs")
                    nc.vector.tensor_copy(bcs[:], bc[:])
                    om = o_pool.tile([C, NT], F32, tag="om")
                    nc.vector.tensor_mul(om[:], av[0:C, :], bcs[:])
                    o = o_pool.tile([C, NT], F32, tag="o")
                    nc.vector.tensor_add(o[:], om[:], xof_sb[:, n0:n1])
                    nc.sync.dma_start(out_d[:, n0:n1], o[:])

                prev = None  # (av, r, nt) awaiting apply
                for nt in range(N_NT):
                    n0, n1 = nt * NT, (nt + 1) * NT
                    av = av_pool.tile([CA, NT], F32)

                    def emit_av(e_t, w, av=av):
                        for j in range(WAVE):
                            mb = WAVE * w + j
                            nc.tensor.matmul(
                                av[:],
                                vT[:, mb * VW : mb * VW + CA],
                                e_t[:, j * NT : (j + 1) * NT],
                                start=(mb == 0),
                                stop=(mb == N_MB - 1),
                            )

                    # S^T matmuls + exp, with the AV accumulation lagging two
                    # waves so the PE never stalls waiting on the current exp.
                    pend = []
                    for w in range(N_MB // WAVE):
                        st = st_pool.tile([MB, WAVE * NT], F32)
                        for j in range(WAVE):
                            mb = WAVE * w + j
                            rg = 64 * j
                            nc.tensor.matmul(
                                st[:, j * NT : (j + 1) * NT],
                                k_rep[rg : rg + D, mb * MB : (mb + 1) * MB],
                                q_rep[rg : rg + D, n0:n1],
                                start=True,
                                stop=True,
                            )
                        e_t = e_pool.tile([MB, WAVE * NT], BF16)
                        nc.scalar.activation(e_t[:], st[:], AF.Exp)
                        pend.append((e_t, w))
                        if len(pend) > 2:
                            emit_av(*pend.pop(0))
                        # normalize of the PREVIOUS tile, mid-stream so the
                        # PE/DVE chain never gates a tile boundary
                        if w == 2 and prev is not None:
                            norm_apply(*prev)
                            prev = None
                    for p in pend:
                        emit_av(*p)
                    prev = (av, norm_recip(av), nt)

                norm_apply(*prev)
    nc.compile()
    return nc


_NC = None


def _get_nc() -> bass.Bass:
    global _NC
    if _NC is None:
        _NC = build_program()
    return _NC


def make_in_maps(x_opt, x_sar, wq, bq, wk, bk, wv, bv, gamma):
    f = np.float32
    x_opt = np.asarray(x_opt, f).reshape(B, C, N)
    x_sar = np.asarray(x_sar, f).reshape(B, C, N)
    g = float(np.asarray(gamma, f).reshape(-1)[0])

    wq_aug = np.concatenate([np.asarray(wq, f).T, np.asarray(bq, f)[None, :]], 0)
    wk_aug = np.concatenate([np.asarray(wk, f).T, np.asarray(bk, f)[None, :]], 0)
    wv_aug = np.zeros((CA, VW), f)
    wv_aug[:C, :C] = np.asarray(wv, f).T * g
    wv_aug[C, :C] = np.asarray(bv, f) * g
    wv_aug[C, C] = 1.0  # denominator column (gamma cancels in the divide)
    wpack = np.ascontiguousarray(
        np.concatenate([wq_aug, wk_aug, wv_aug], axis=1).astype(NP_BF16)
    )

    ones_n = np.ones((1, N), f)
    maps = []
    for core in range(NCORES):
        b, h = divmod(core, 2)
        xo = x_opt[b, :, h * NL : (h + 1) * NL]
        xo_bf = np.ascontiguousarray(
            np.concatenate([xo, ones_n[:, :NL]], axis=0).astype(NP_BF16)
        )
        xs_bf = np.ascontiguousarray(
            np.concatenate([x_sar[b], ones_n], axis=0).astype(NP_BF16)
        )
        maps.append(
            {
                "xo_bf": xo_bf,
                "xs_bf": xs_bf,
                "xof": np.ascontiguousarray(xo),
                "wpack": wpack,
            }
        )
    return maps


def assemble_out(results) -> np.ndarray:
    out = np.empty((B, C, N), np.float32)
    for core in range(NCORES):
        b, h = divmod(core, 2)
        out[b, :, h * NL : (h + 1) * NL] = results[core]["out"]
    return out.reshape(B, C, HH, WW)


def kernel(**inputs) -> np.ndarray:
    nc = _get_nc()
    maps = make_in_maps(**inputs)
    res = run_bass_kernel_spmd(nc, maps, list(range(NCORES)))
    return assemble_out(res.results)


# revision 31
# speedup vs baseline: 1.2423x; 1.1927x over previous
"""Trainium2 Bass kernel for CrossModalFusion (B=4, C=64, H=W=64, N=4096).

Reference computation (per sample b, with x reshaped to [C, N]):
    q = wq @ xo + bq          [8, N]
    k = wk @ xs + bk          [8, N]
    v = wv @ xs + bv          [64, N]
    S[n, m]  = q[:, n] . k[:, m]
    attn     = softmax_m(S)
    out      = gamma * (v @ attn^T) + x_opt

Sharding: 8 cores = 4 batch samples x 2 halves of the query (n) axis.
Each core computes output rows [64, 2048] for its (sample, n-half); no
cross-core communication is needed.

Per-core dataflow (bf16 fast path):
  - biases are folded into augmented weights on the host (ones-row trick);
    gamma is folded into the v columns of wv (the denominator column stays
    unscaled, so gamma cancels out of the normalization exactly when the
    kernel divides by the accumulated denominator).
  - all PE-heavy matmuls run in bf16 (observed ~3x faster per 512-col
    matmul than fp32r on TRN2): q/k/v projections, S^T score matmuls and
    the attention*V accumulation.  Scores accumulate in fp32 PSUM; exp
    runs on the scalar (ACT) engine reading PSUM and writing bf16 SBUF.
  - scores are computed TRANSPOSED (S^T[m, n]) so the exp'd scores feed
    the attention*V matmul directly as the moving operand.  v^T carries an
    extra ones column, so the AV matmul's row 64 accumulates
    sum_m exp(S[n, m]) -- the softmax denominator comes out of the same
    accumulation for free.  No max-subtraction is needed: scores are O(3).
  - q/k are replicated at partition offsets 0 and 64 so the rank-8 S^T
    matmuls alternate PE row groups (stationary double-buffering).
  - per n-tile of 512: accumulate all 32 m-blocks, then normalize via
    reciprocal_approx_fast (custom DVE op, ~5x faster than the iterative
    divide), a PE ones-broadcast matmul, and two DVE element-wise ops.
    The normalize for tile t is emitted in the middle of tile t+1's wave
    loop so it never stalls the PE at tile boundaries.
  - residual x_opt is DMA'd separately in fp32 (off the critical path) so
    the gamma=0 output is bit-accurate to x_opt up to the fp32 add.
"""

import os
import sys

import numpy as np

for _p in ("/opt/trn_rl_repo", "/root/.axon_site/_ro/trn_rl_repo"):
    if os.path.isdir(_p) and _p not in sys.path:
        sys.path.insert(0, _p)

import ml_dtypes

import concourse.bass as bass
import concourse.mybir as mybir
import concourse.tile as tile
from concourse import bacc
from concourse.bass_utils import run_bass_kernel_spmd

F32 = mybir.dt.float32
F32R = mybir.dt.float32r
BF16 = mybir.dt.bfloat16
FP8 = mybir.dt.float8e4
AF = mybir.ActivationFunctionType
NP_BF16 = np.dtype(ml_dtypes.bfloat16)
EXP_BIAS = -1.5  # exp(s + EXP_BIAS): keeps e_t within fp8e4 range; the
                 # common factor cancels between numerator and denominator

B, C, HH, WW = 4, 64, 64, 64
N = HH * WW            # 4096 key/query positions
D = 8                  # q/k channel count
CA = C + 1             # augmented channel dim (ones row / denominator row)
VW = 80                # padded v^T block width (DoubleRow k-tile step %16==0)
WCOLS = D + D + VW     # packed weight buffer width (wq | wk | wv')
NCORES = 8
NL = N // 2            # query rows per core
NT = 512               # n-tile (PSUM bank width in fp32)
MB = 128               # m-block (PE partition width)
N_NT = NL // NT        # 4 n-tiles per core
N_MB = N // MB         # 32 m-blocks
WAVE = 2               # m-blocks exp'd per ACT instruction


def build_program(repeat: int = 1) -> bass.Bass:
    # Bacc (not raw Bass): its compile() pass splits multi-semaphore waits
    # and moves matmul waits onto LDWEIGHTS, which this walrus build requires.
    nc = bacc.Bacc("TRN2", target_bir_lowering=False, num_devices=NCORES)
    xo_d = nc.declare_dram_parameter("xo_bf", [CA, NL], BF16, isOutput=False)
    xs_d = nc.declare_dram_parameter("xs_bf", [CA, N], BF16, isOutput=False)
    xof_d = nc.declare_dram_parameter("xof", [C, NL], F32, isOutput=False)
    w_d = nc.declare_dram_parameter("wpack", [CA, WCOLS], BF16, isOutput=False)
    out_d = nc.declare_dram_parameter("out", [C, NL], F32, isOutput=True)

    with tile.TileContext(nc) as tc:
      for _rep in range(repeat):
        with tc.tile_pool(name="const", bufs=1) as cp:
            # --- input DMAs, spread across per-engine DGE queues ---
            w_sb = cp.tile([CA, WCOLS], BF16)
            nc.sync.dma_start(w_sb[:], w_d[:])
            xo_bf = cp.tile([CA, NL], BF16)
            for j in range(2):
                nc.sync.dma_start(
                    xo_bf[:, j * 1024 : (j + 1) * 1024],
                    xo_d[:, j * 1024 : (j + 1) * 1024],
                )
            xs_bf = cp.tile([CA, N], BF16)
            for j in range(4):
                nc.gpsimd.dma_start(
                    xs_bf[:, j * 1024 : (j + 1) * 1024],
                    xs_d[:, j * 1024 : (j + 1) * 1024],
                )
            xof_sb = cp.tile([C, NL], F32)
            ones_sb = cp.tile([1, C], BF16)
            nc.vector.memset(ones_sb[:], 1.0)
            ebias_sb = cp.tile([MB, 1], F32)
            nc.vector.memset(ebias_sb[:], EXP_BIAS)
            wq_sb = w_sb[:, 0:D]
            wk_sb = w_sb[:, D : 2 * D]
            wv_sb = w_sb[:, 2 * D : 2 * D + VW]

            # q/k at partition offsets 0 and 64 (PE row groups for the
            # alternating rank-8 S^T matmuls); v^T with denominator column.
            q_rep = cp.tile([64 + D, NL], BF16)
            k_rep = cp.tile([64 + D, N], BF16)
            # v^T blocks in fp8e4: feeds the DoubleRow attention*V matmul
            vT = cp.tile([MB, N_MB, VW], FP8)

            with tc.tile_pool(name="pre_ps", bufs=3, space="PSUM") as pp:
                def emit_q(j):
                    qp = pp.tile([D, NT], F32, tag="qk_ps")
                    nc.tensor.matmul(
                        qp[:], wq_sb, xo_bf[:, j * NT : (j + 1) * NT],
                        start=True, stop=True,
                    )
                    sl = q_rep[0:D, j * NT : (j + 1) * NT]
                    # q casts on ACT (idle in preamble), k casts on DVE
                    nc.scalar.copy(sl, qp[:])
                    # partition-offset replica: engines are lane-aligned, so
                    # this must be a DMA (gpsimd DGE queue)
                    nc.gpsimd.dma_start(q_rep[64 : 64 + D, j * NT : (j + 1) * NT], sl)

                def emit_k(j):
                    kp = pp.tile([D, NT], F32, tag="qk_ps")
                    nc.tensor.matmul(
                        kp[:], wk_sb, xs_bf[:, j * NT : (j + 1) * NT],
                        start=True, stop=True,
                    )
                    sl = k_rep[0:D, j * NT : (j + 1) * NT]
                    nc.vector.tensor_copy(sl, kp[:])
                    nc.gpsimd.dma_start(k_rep[64 : 64 + D, j * NT : (j + 1) * NT], sl)

                def emit_v(g):
                    # 4 m-blocks per PSUM tile, one cast for all 4
                    vp = pp.tile([MB, 4 * VW], F32, tag="vp_ps")
                    for t in range(4):
                        mb = 4 * g + t
                        nc.tensor.matmul(
                            vp[:, t * VW : (t + 1) * VW],
                            xs_bf[:, mb * MB : (mb + 1) * MB],
                            wv_sb,
                            start=True, stop=True,
                        )
                    nc.vector.tensor_copy(vT[:, 4 * g : 4 * (g + 1), :], vp[:])

                # ordered by first use in the wave loop
                emit_q(0); emit_q(1); emit_k(0); emit_k(1)
                emit_v(0); emit_v(1)
                emit_q(2); emit_q(3); emit_k(2); emit_k(3)
                emit_v(2); emit_v(3)
                emit_k(4); emit_k(5); emit_v(4); emit_v(5)
                emit_k(6); emit_k(7); emit_v(6); emit_v(7)

            # residual; needed late (first normalize), queued after replicas
            for j in range(2):
                nc.gpsimd.dma_start(
                    xof_sb[:, j * 1024 : (j + 1) * 1024],
                    xof_d[:, j * 1024 : (j + 1) * 1024],
                )

            with (
                tc.tile_pool(name="st_ps", bufs=2, space="PSUM") as st_pool,
                tc.tile_pool(name="av_ps", bufs=2, space="PSUM") as av_pool,
                tc.tile_pool(name="bc_ps", bufs=1, space="PSUM") as bc_pool,
                tc.tile_pool(name="e_sb", bufs=6) as e_pool,
                tc.tile_pool(name="o_sb", bufs=2) as o_pool,
                tc.tile_pool(name="sm_sb", bufs=2) as sm_pool,
            ):
                def norm_recip(av):
                    # softmax denominator -> reciprocal (row 64 of av)
                    r = sm_pool.tile([1, NT], F32, tag="r")
                    nc.vector.reciprocal(r[:], av[C:CA, :])
                    # bf16 copy feeds the broadcast matmul (bf16 is ~2x
                    # faster than the fp32 LOW/HIGH pair on the drain path)
                    rb = sm_pool.tile([1, NT], BF16, tag="rb")
                    nc.vector.tensor_copy(rb[:], r[:])
                    return rb

                def norm_apply(av, rb, nt):
                    n0, n1 = nt * NT, (nt + 1) * NT
                    # broadcast 1/denom across the 64 channel partitions via
                    # a rank-1 ones matmul
                    bc = bc_pool.tile([C, NT], F32)
                    nc.tensor.matmul(bc[:], ones_sb[:], rb[:], start=True, stop=True)
                    bcs = o_pool.tile([C, NT], F32, tag="bcs")
                    nc.vector.tensor_copy(bcs[:], bc[:])
                    om = o_pool.tile([C, NT], F32, tag="om")
                    nc.vector.tensor_mul(om[:], av[0:C, :], bcs[:])
                    o = o_pool.tile([C, NT], F32, tag="o")
                    nc.vector.tensor_add(o[:], om[:], xof_sb[:, n0:n1])
                    nc.sync.dma_start(out_d[:, n0:n1], o[:])

                prev = None  # (av, r, nt) awaiting apply
                for nt in range(N_NT):
                    n0, n1 = nt * NT, (nt + 1) * NT
                    av = av_pool.tile([CA, NT], F32)

                    def emit_av(e_t, w, av=av):
                        # fp8 DoubleRow: one matmul accumulates BOTH m-blocks
                        # of the wave (2 contraction k-tiles at 2 rows/cycle)
                        nc.tensor.matmul(
                            av[:],
                            vT[:, WAVE * w : WAVE * (w + 1), 0:CA],
                            e_t[:],
                            perf_mode=mybir.MatmulPerfMode.DoubleRow,
                            start=(w == 0),
                            stop=(w == N_MB // WAVE - 1),
                        )

                    # S^T matmuls + exp, with the AV accumulation lagging two
                    # waves so the PE never stalls waiting on the current exp.
                    pend = []
                    for w in range(N_MB // WAVE):
                        st = st_pool.tile([MB, WAVE, NT], F32)
                        for j in range(WAVE):
                            mb = WAVE * w + j
                            rg = 64 * j
                            nc.tensor.matmul(
                                st[:, j, :],
                                k_rep[rg : rg + D, mb * MB : (mb + 1) * MB],
                                q_rep[rg : rg + D, n0:n1],
                                start=True,
                                stop=True,
                            )
                        e_t = e_pool.tile([MB, WAVE, NT], FP8)
                        nc.scalar.activation(e_t[:], st[:], AF.Exp, bias=ebias_sb[:])
                        pend.append((e_t, w))
                        if len(pend) > 2:
                            emit_av(*pend.pop(0))
                        # normalize of the PREVIOUS tile, mid-stream so the
                        # PE/DVE chain never gates a tile boundary
                        if w == 2 and prev is not None:
                            norm_apply(*prev)
                            prev = None
                    for p in pend:
                        emit_av(*p)
                    prev = (av, norm_recip(av), nt)

                norm_apply(*prev)
    nc.compile()
    return nc


_NC = None


def _get_nc() -> bass.Bass:
    global _NC
    if _NC is None:
        _NC = build_program()
    return _NC


def make_in_maps(x_opt, x_sar, wq, bq, wk, bk, wv, bv, gamma):
    f = np.float32
    x_opt = np.asarray(x_opt, f).reshape(B, C, N)
    x_sar = np.asarray(x_sar, f).reshape(B, C, N)
    g = float(np.asarray(gamma, f).reshape(-1)[0])

    wq_aug = np.concatenate([np.asarray(wq, f).T, np.asarray(bq, f)[None, :]], 0)
    wk_aug = np.concatenate([np.asarray(wk, f).T, np.asarray(bk, f)[None, :]], 0)
    wv_aug = np.zeros((CA, VW), f)
    wv_aug[:C, :C] = np.asarray(wv, f).T * g
    wv_aug[C, :C] = np.asarray(bv, f) * g
    wv_aug[C, C] = 1.0  # denominator column (gamma cancels in the divide)
    # columns C+1..VW-1 stay zero (DoubleRow k-tile alignment padding)
    wpack = np.ascontiguousarray(
        np.concatenate([wq_aug, wk_aug, wv_aug], axis=1).astype(NP_BF16)
    )

    ones_n = np.ones((1, N), f)
    maps = []
    for core in range(NCORES):
        b, h = divmod(core, 2)
        xo = x_opt[b, :, h * NL : (h + 1) * NL]
        xo_bf = np.ascontiguousarray(
            np.concatenate([xo, ones_n[:, :NL]], axis=0).astype(NP_BF16)
        )
        xs_bf = np.ascontiguousarray(
            np.concatenate([x_sar[b], ones_n], axis=0).astype(NP_BF16)
        )
        maps.append(
            {
                "xo_bf": xo_bf,
                "xs_bf": xs_bf,
                "xof": np.ascontiguousarray(xo),
                "wpack": wpack,
            }
        )
    return maps


def assemble_out(results) -> np.ndarray:
    out = np.empty((B, C, N), np.float32)
    for core in range(NCORES):
        b, h = divmod(core, 2)
        out[b, :, h * NL : (h + 1) * NL] = results[core]["out"]
    return out.reshape(B, C, HH, WW)


def kernel(**inputs) -> np.ndarray:
    nc = _get_nc()
    maps = make_in_maps(**inputs)
    res = run_bass_kernel_spmd(nc, maps, list(range(NCORES)))
    return assemble_out(res.results)


# revision 38
# speedup vs baseline: 1.2458x; 1.0028x over previous
"""Trainium2 Bass kernel for CrossModalFusion (B=4, C=64, H=W=64, N=4096).

Reference computation (per sample b, with x reshaped to [C, N]):
    q = wq @ xo + bq          [8, N]
    k = wk @ xs + bk          [8, N]
    v = wv @ xs + bv          [64, N]
    S[n, m]  = q[:, n] . k[:, m]
    attn     = softmax_m(S)
    out      = gamma * (v @ attn^T) + x_opt

Sharding: 8 cores = 4 batch samples x 2 halves of the query (n) axis.
Each core computes output rows [64, 2048] for its (sample, n-half); no
cross-core communication is needed.

Per-core dataflow (bf16 fast path):
  - biases are folded into augmented weights on the host (ones-row trick);
    gamma is folded into the v columns of wv (the denominator column stays
    unscaled, so gamma cancels out of the normalization exactly when the
    kernel divides by the accumulated denominator).
  - all PE-heavy matmuls run in bf16 (observed ~3x faster per 512-col
    matmul than fp32r on TRN2): q/k/v projections, S^T score matmuls and
    the attention*V accumulation.  Scores accumulate in fp32 PSUM; exp
    runs on the scalar (ACT) engine reading PSUM and writing bf16 SBUF.
  - scores are computed TRANSPOSED (S^T[m, n]) so the exp'd scores feed
    the attention*V matmul directly as the moving operand.  v^T carries an
    extra ones column, so the AV matmul's row 64 accumulates
    sum_m exp(S[n, m]) -- the softmax denominator comes out of the same
    accumulation for free.  No max-subtraction is needed: scores are O(3).
  - q/k are replicated at partition offsets 0 and 64 so the rank-8 S^T
    matmuls alternate PE row groups (stationary double-buffering).
  - per n-tile of 512: accumulate all 32 m-blocks, then normalize via
    reciprocal_approx_fast (custom DVE op, ~5x faster than the iterative
    divide), a PE ones-broadcast matmul, and two DVE element-wise ops.
    The normalize for tile t is emitted in the middle of tile t+1's wave
    loop so it never stalls the PE at tile boundaries.
  - residual x_opt is DMA'd separately in fp32 (off the critical path) so
    the gamma=0 output is bit-accurate to x_opt up to the fp32 add.
"""

import os
import sys

import numpy as np

for _p in ("/opt/trn_rl_repo", "/root/.axon_site/_ro/trn_rl_repo"):
    if os.path.isdir(_p) and _p not in sys.path:
        sys.path.insert(0, _p)

import ml_dtypes

import concourse.bass as bass
import concourse.mybir as mybir
import concourse.tile as tile
from concourse import bacc
from concourse.bass_utils import run_bass_kernel_spmd

F32 = mybir.dt.float32
F32R = mybir.dt.float32r
BF16 = mybir.dt.bfloat16
FP8 = mybir.dt.float8e4
AF = mybir.ActivationFunctionType
NP_BF16 = np.dtype(ml_dtypes.bfloat16)
EXP_BIAS = -1.5  # exp(s + EXP_BIAS): keeps e_t within fp8e4 range; the
                 # common factor cancels between numerator and denominator

B, C, HH, WW = 4, 64, 64, 64
N = HH * WW            # 4096 key/query positions
D = 8                  # q/k channel count
CA = C + 1             # augmented channel dim (ones row / denominator row)
VW = 80                # padded v^T block width (DoubleRow k-tile step %16==0)
WCOLS = D + D + VW     # packed weight buffer width (wq | wk | wv')
NCORES = 8
NL = N // 2            # query rows per core
NT = 512               # n-tile (PSUM bank width in fp32)
MB = 128               # m-block (PE partition width)
N_NT = NL // NT        # 4 n-tiles per core
N_MB = N // MB         # 32 m-blocks
WAVE = 2               # m-blocks exp'd per ACT instruction


def build_program(repeat: int = 1) -> bass.Bass:
    # Bacc (not raw Bass): its compile() pass splits multi-semaphore waits
    # and moves matmul waits onto LDWEIGHTS, which this walrus build requires.
    nc = bacc.Bacc("TRN2", target_bir_lowering=False, num_devices=NCORES)
    xo_d = nc.declare_dram_parameter("xo_bf", [CA, NL], BF16, isOutput=False)
    xs_d = nc.declare_dram_parameter("xs_bf", [CA, N], BF16, isOutput=False)
    xof_d = nc.declare_dram_parameter("xof", [C, NL], F32, isOutput=False)
    w_d = nc.declare_dram_parameter("wpack", [CA, WCOLS], BF16, isOutput=False)
    out_d = nc.declare_dram_parameter("out", [C, NL], F32, isOutput=True)

    with tile.TileContext(nc) as tc:
      for _rep in range(repeat):
        with tc.tile_pool(name="const", bufs=1) as cp:
            # --- input DMAs, spread across per-engine DGE queues ---
            w_sb = cp.tile([CA, WCOLS], BF16)
            nc.sync.dma_start(w_sb[:], w_d[:])
            xo_bf = cp.tile([CA, NL], BF16)
            for j in range(2):
                nc.sync.dma_start(
                    xo_bf[:, j * 1024 : (j + 1) * 1024],
                    xo_d[:, j * 1024 : (j + 1) * 1024],
                )
            xs_bf = cp.tile([CA, N], BF16)

            def load_xs(j):
                nc.gpsimd.dma_start(
                    xs_bf[:, j * 1024 : (j + 1) * 1024],
                    xs_d[:, j * 1024 : (j + 1) * 1024],
                )

            load_xs(0)
            xof_sb = cp.tile([C, NL], F32)
            ones_sb = cp.tile([1, C], BF16)
            nc.vector.memset(ones_sb[:], 1.0)
            ebias_sb = cp.tile([MB, 1], F32)
            nc.vector.memset(ebias_sb[:], EXP_BIAS)
            wq_sb = w_sb[:, 0:D]
            wk_sb = w_sb[:, D : 2 * D]
            wv_sb = w_sb[:, 2 * D : 2 * D + VW]

            # q/k at partition offsets 0 and 64 (PE row groups for the
            # alternating rank-8 S^T matmuls); v^T with denominator column.
            q_rep = cp.tile([64 + D, NL], BF16)
            k_rep = cp.tile([64 + D, N], BF16)
            # v^T blocks in fp8e4: feeds the DoubleRow attention*V matmul
            vT = cp.tile([MB, N_MB, VW], FP8)

            with tc.tile_pool(name="pre_ps", bufs=3, space="PSUM") as pp:
                def emit_q(j):
                    qp = pp.tile([D, NT], F32, tag="qk_ps")
                    nc.tensor.matmul(
                        qp[:], wq_sb, xo_bf[:, j * NT : (j + 1) * NT],
                        start=True, stop=True,
                    )
                    sl = q_rep[0:D, j * NT : (j + 1) * NT]
                    # early q casts on ACT (idle in preamble; a cast there
                    # would delay the first exp for late ones), rest on DVE
                    if j < 2:
                        nc.scalar.copy(sl, qp[:])
                    else:
                        nc.vector.tensor_copy(sl, qp[:])
                    # partition-offset replica: engines are lane-aligned, so
                    # this must be a DMA (gpsimd DGE queue)
                    nc.gpsimd.dma_start(q_rep[64 : 64 + D, j * NT : (j + 1) * NT], sl)

                def emit_k(j):
                    kp = pp.tile([D, NT], F32, tag="qk_ps")
                    nc.tensor.matmul(
                        kp[:], wk_sb, xs_bf[:, j * NT : (j + 1) * NT],
                        start=True, stop=True,
                    )
                    sl = k_rep[0:D, j * NT : (j + 1) * NT]
                    nc.vector.tensor_copy(sl, kp[:])
                    nc.gpsimd.dma_start(k_rep[64 : 64 + D, j * NT : (j + 1) * NT], sl)

                def emit_v(g):
                    # 4 m-blocks per PSUM tile, one cast for all 4
                    vp = pp.tile([MB, 4 * VW], F32, tag="vp_ps")
                    for t in range(4):
                        mb = 4 * g + t
                        nc.tensor.matmul(
                            vp[:, t * VW : (t + 1) * VW],
                            xs_bf[:, mb * MB : (mb + 1) * MB],
                            wv_sb,
                            start=True, stop=True,
                        )
                    nc.vector.tensor_copy(vT[:, 4 * g : 4 * (g + 1), :], vp[:])

                # ordered by first use in the wave loop; xs chunk DMAs are
                # interleaved so the replica DMAs behind them land early
                emit_q(0); emit_q(1); emit_k(0)
                load_xs(1)
                emit_k(1); emit_v(0); emit_v(1)
                load_xs(2)
                emit_q(2); emit_q(3); emit_k(2); emit_k(3)
                load_xs(3)
                emit_v(2); emit_v(3)
                emit_k(4); emit_k(5); emit_v(4); emit_v(5)
                emit_k(6); emit_k(7); emit_v(6); emit_v(7)

            # residual; needed late (first normalize), queued after replicas
            for j in range(2):
                nc.gpsimd.dma_start(
                    xof_sb[:, j * 1024 : (j + 1) * 1024],
                    xof_d[:, j * 1024 : (j + 1) * 1024],
                )

            with (
                tc.tile_pool(name="st_ps", bufs=2, space="PSUM") as st_pool,
                tc.tile_pool(name="av_ps", bufs=2, space="PSUM") as av_pool,
                tc.tile_pool(name="bc_ps", bufs=1, space="PSUM") as bc_pool,
                tc.tile_pool(name="e_sb", bufs=6) as e_pool,
                tc.tile_pool(name="o_sb", bufs=2) as o_pool,
                tc.tile_pool(name="sm_sb", bufs=2) as sm_pool,
            ):
                def norm_recip(av):
                    # softmax denominator -> reciprocal (row 64 of av)
                    r = sm_pool.tile([1, NT], F32, tag="r")
                    nc.vector.reciprocal(r[:], av[C:CA, :])
                    # bf16 copy feeds the broadcast matmul (bf16 is ~2x
                    # faster than the fp32 LOW/HIGH pair on the drain path)
                    rb = sm_pool.tile([1, NT], BF16, tag="rb")
                    nc.vector.tensor_copy(rb[:], r[:])
                    return rb

                def norm_apply(av, rb, nt):
                    n0, n1 = nt * NT, (nt + 1) * NT
                    # broadcast 1/denom across the 64 channel partitions via
                    # a rank-1 ones matmul
                    bc = bc_pool.tile([C, NT], F32)
                    nc.tensor.matmul(bc[:], ones_sb[:], rb[:], start=True, stop=True)
                    bcs = o_pool.tile([C, NT], F32, tag="bcs")
                    nc.vector.tensor_copy(bcs[:], bc[:])
                    om = o_pool.tile([C, NT], F32, tag="om")
                    nc.vector.tensor_mul(om[:], av[0:C, :], bcs[:])
                    o = o_pool.tile([C, NT], F32, tag="o")
                    nc.vector.tensor_add(o[:], om[:], xof_sb[:, n0:n1])
                    nc.sync.dma_start(out_d[:, n0:n1], o[:])

                prev = None  # (av, r, nt) awaiting apply
                for nt in range(N_NT):
                    n0, n1 = nt * NT, (nt + 1) * NT
                    av = av_pool.tile([CA, NT], F32)

                    def emit_av(e_t, w, av=av):
                        # fp8 DoubleRow: one matmul accumulates BOTH m-blocks
                        # of the wave (2 contraction k-tiles at 2 rows/cycle)
                        nc.tensor.matmul(
                            av[:],
                            vT[:, WAVE * w : WAVE * (w + 1), 0:CA],
                            e_t[:],
                            perf_mode=mybir.MatmulPerfMode.DoubleRow,
                            start=(w == 0),
                            stop=(w == N_MB // WAVE - 1),
                        )

                    # S^T matmuls + exp, with the AV accumulation lagging two
                    # waves so the PE never stalls waiting on the current exp.
                    pend = []
                    for w in range(N_MB // WAVE):
                        st = st_pool.tile([MB, WAVE, NT], F32)
                        for j in range(WAVE):
                            mb = WAVE * w + j
                            rg = 64 * j
                            nc.tensor.matmul(
                                st[:, j, :],
                                k_rep[rg : rg + D, mb * MB : (mb + 1) * MB],
                                q_rep[rg : rg + D, n0:n1],
                                start=True,
                                stop=True,
                            )
                        e_t = e_pool.tile([MB, WAVE, NT], FP8)
                        nc.scalar.activation(e_t[:], st[:], AF.Exp, bias=ebias_sb[:])
                        pend.append((e_t, w))
                        # lag-2 in steady state, drained to lag-1 at the tile
                        # end so the boundary tail is one AV matmul, not two
                        while len(pend) > (2 if w < N_MB // WAVE - 1 else 1):
                            emit_av(*pend.pop(0))
                        # normalize of the PREVIOUS tile, mid-stream so the
                        # PE/DVE chain never gates a tile boundary
                        if w == 2 and prev is not None:
                            norm_apply(*prev)
                            prev = None
                    for p in pend:
                        emit_av(*p)
                    prev = (av, norm_recip(av), nt)

                norm_apply(*prev)
    nc.compile()
    return nc


_NC = None


def _get_nc() -> bass.Bass:
    global _NC
    if _NC is None:
        _NC = build_program()
    return _NC


def make_in_maps(x_opt, x_sar, wq, bq, wk, bk, wv, bv, gamma):
    f = np.float32
    x_opt = np.asarray(x_opt, f).reshape(B, C, N)
    x_sar = np.asarray(x_sar, f).reshape(B, C, N)
    g = float(np.asarray(gamma, f).reshape(-1)[0])

    wq_aug = np.concatenate([np.asarray(wq, f).T, np.asarray(bq, f)[None, :]], 0)
    wk_aug = np.concatenate([np.asarray(wk, f).T, np.asarray(bk, f)[None, :]], 0)
    wv_aug = np.zeros((CA, VW), f)
    wv_aug[:C, :C] = np.asarray(wv, f).T * g
    wv_aug[C, :C] = np.asarray(bv, f) * g
    wv_aug[C, C] = 1.0  # denominator column (gamma cancels in the divide)
    # columns C+1..VW-1 stay zero (DoubleRow k-tile alignment padding)
    wpack = np.ascontiguousarray(
        np.concatenate([wq_aug, wk_aug, wv_aug], axis=1).astype(NP_BF16)
    )

    ones_n = np.ones((1, N), f)
    maps = []
    for core in range(NCORES):
        b, h = divmod(core, 2)
        xo = x_opt[b, :, h * NL : (h + 1) * NL]
        xo_bf = np.ascontiguousarray(
            np.concatenate([xo, ones_n[:, :NL]], axis=0).astype(NP_BF16)
        )
        xs_bf = np.ascontiguousarray(
            np.concatenate([x_sar[b], ones_n], axis=0).astype(NP_BF16)
        )
        maps.append(
            {
                "xo_bf": xo_bf,
                "xs_bf": xs_bf,
                "xof": np.ascontiguousarray(xo),
                "wpack": wpack,
            }
        )
    return maps


def assemble_out(results) -> np.ndarray:
    out = np.empty((B, C, N), np.float32)
    for core in range(NCORES):
        b, h = divmod(core, 2)
        out[b, :, h * NL : (h + 1) * NL] = results[core]["out"]
    return out.reshape(B, C, HH, WW)


def kernel(**inputs) -> np.ndarray:
    nc = _get_nc()
    maps = make_in_maps(**inputs)
    res = run_bass_kernel_spmd(nc, maps, list(range(NCORES)))
    return assemble_out(res.results)
